# revision 1
# baseline (speedup 1.0000x reference)
"""Gaussian RBF kernel-mean loss on 8 Trainium2 NeuronCores.

Computes mean(exp(-||x_i - y_j||^2 / 2)) over all (i, j) pairs for
x, y of shape [8192, 256] fp32.

Math used on device (per core, rows of x sharded 1024/core):
    exp(-d2/2) = exp(x.y - 0.5||x||^2) * exp(-0.5||y||^2)
so each output tile is:
    E  = exp(psum + bias_m)        # ACT, bias is per-partition -0.5||x_m||^2
    acc += E * ey_n                # DVE scalar_tensor_tensor + accum_out,
                                   # ey is the column factor exp(-0.5||y_n||^2)
where psum = x @ y.T accumulated over K=256 in two 128-chunks on the PE.
Per-core partial sums [128, NTILES] are DMA'd out; the host adds the
8 * 128 * NTILES partials and divides by N*M.

Host-side prep (outside HW-timed kernel): transpose/cast x,y to bf16
[K, *] layout so the contraction dim lands on SBUF partitions, plus the
tiny O(N*K) row-norm computations.

Toolchain constraint: this walrus build accepts at most ONE sync wait
per compute instruction. The kernel is therefore a strict
PE -> ACT -> DVE pipeline; slot-recycle WAR waits and DMA-arrival waits
are absorbed by tiny same-engine "observer" ops (LDWEIGHTS on PE,
scalar copies on ACT/DVE) whose single wait subsumes the would-be
second wait of the real instructions.
"""

import numpy as np
import ml_dtypes

N = 8192          # rows of x
M = 8192          # rows of y
K = 256           # feature dim
NCORES = 8
MPC = N // NCORES        # 1024 rows of x per core
P = 128                  # partitions
KO = K // P              # 2 k-chunks
MB = MPC // P            # 8 m-blocks per core
NG_W = 2048              # columns per psum tile (4 banks)
NG = M // NG_W           # 4 n-groups
NS_W = 512               # matmul free width (1 psum bank)
NS = NG_W // NS_W        # 4
NTILES = MB * NG         # 32 output tiles per core
CHUNK = M // 4           # DMA column chunk for yt/ey

_cached = {}
_last_in_maps = None


def _build():
    import concourse.bass as bass
    import concourse.tile as tile
    import concourse.mybir as mybir
    from contextlib import ExitStack

    fp32 = mybir.dt.float32
    bf16 = mybir.dt.bfloat16

    nc = bass.Bass(trn_type="TRN2")
    xt = nc.dram_tensor("xt", [K, MPC], bf16, kind="ExternalInput")
    yt = nc.dram_tensor("yt", [K, M], bf16, kind="ExternalInput")
    xb = nc.dram_tensor("xb", [P, MB], fp32, kind="ExternalInput")
    ey = nc.dram_tensor("ey", [P, M], bf16, kind="ExternalInput")
    stats = nc.dram_tensor("stats", [P, NTILES], fp32, kind="ExternalOutput")

    xt_v = xt.ap().rearrange("(ko p) m -> p ko m", p=P)
    yt_v = yt.ap().rearrange("(ko p) n -> p ko n", p=P)

    with ExitStack() as ctx:
        tc = ctx.enter_context(tile.TileContext(nc))
        singles = ctx.enter_context(tc.tile_pool(name="singles", bufs=1))
        psum_pool = ctx.enter_context(
            tc.tile_pool(name="psum", bufs=2, space="PSUM")
        )
        e_pool = ctx.enter_context(tc.tile_pool(name="e", bufs=4))
        sc_pool = ctx.enter_context(tc.tile_pool(name="sc", bufs=3))

        xt_sb = singles.tile([P, KO, MPC], bf16)
        yt_sb = singles.tile([P, KO, M], bf16)
        ey_sb = singles.tile([P, M], bf16)
        xb_sb = singles.tile([P, MB], fp32)
        st_sb = singles.tile([P, NTILES], fp32)
        warm = singles.tile([P, 1], fp32)
        warmsc = singles.tile([P, NTILES // 2 + 1], fp32)
        dvew = singles.tile([P, NTILES // 2 + 1], bf16)

        nc.sync.dma_start(out=xt_sb, in_=xt_v)
        nc.sync.dma_start(out=xb_sb, in_=xb.ap())
        # PE observer for the xt DMA queue (no PSUM write -> no bank WAW)
        nc.tensor.ldweights(weights=xt_sb[:, 0, 0:P])
        # ACT warmup: loads the exp table set AND observes the xb DMA queue,
        # so no later Exp carries the table-load's extra sync wait.
        nc.scalar.activation(
            out=warm, in_=xb_sb[:, 0:1], func=mybir.ActivationFunctionType.Exp
        )
        # input column chunks (yt for PE, ey for DVE)
        for g in range(4):
            cs = slice(g * CHUNK, (g + 1) * CHUNK)
            nc.sync.dma_start(out=yt_sb[:, :, cs], in_=yt_v[:, :, cs])
            nc.sync.dma_start(out=ey_sb[:, cs], in_=ey.ap()[:, cs])

        e_list = []
        sc_list = []
        t = 0
        for mb in range(MB):
            ms = slice(mb * P, (mb + 1) * P)
            for ng in range(NG):
                if mb == 0:
                    g = ng
                    c0 = g * CHUNK
                    if g > 0:
                        # PE observer: absorb the yt chunk-g DMA wait
                        nc.tensor.ldweights(weights=yt_sb[:, 0, c0 : c0 + P])
                    # DVE observer: absorb the ey chunk-g DMA wait
                    eyw = singles.tile([P, 1], bf16, name=f"eyw{g}")
                    nc.vector.tensor_copy(out=eyw, in_=ey_sb[:, c0 : c0 + 1])
                if t >= 2:
                    # PE observer: absorb the psum-slot-recycle wait
                    # (ACT finished exp of tile t-2).
                    nc.tensor.ldweights(weights=e_list[t - 2][:, 0:P])
                psum = psum_pool.tile([P, NG_W], fp32)
                for k in range(KO):
                    for ns in range(NS):
                        c0 = ng * NG_W + ns * NS_W
                        nc.tensor.matmul(
                            psum[:, ns * NS_W : (ns + 1) * NS_W],
                            xt_sb[:, k, ms],
                            yt_sb[:, k, c0 : c0 + NS_W],
                            start=(k == 0),
                            stop=(k == KO - 1),
                        )
                if t >= 2 and t % 2 == 0:
                    # ACT observer: absorb the e-slot-recycle WAR wait by
                    # observing DVE progress through the stats column it
                    # wrote two tiles ago.
                    w = t // 2
                    nc.scalar.copy(
                        out=warmsc[:, w : w + 1], in_=st_sb[:, t - 2 : t - 1]
                    )
                e_t = e_pool.tile([P, NG_W], bf16)
                nc.scalar.activation(
                    out=e_t,
                    in_=psum,
                    func=mybir.ActivationFunctionType.Exp,
                    bias=xb_sb[:, mb : mb + 1],
                    scale=1.0,
                )
                sc = sc_pool.tile([P, NG_W], bf16)
                nc.vector.scalar_tensor_tensor(
                    out=sc,
                    in0=e_t,
                    scalar=1.0,
                    in1=ey_sb[:, ng * NG_W : (ng + 1) * NG_W],
                    op0=mybir.AluOpType.mult,
                    op1=mybir.AluOpType.mult,
                    accum_out=st_sb[:, t : t + 1],
                )
                e_list.append(e_t)
                sc_list.append(sc)
                t += 1

        nc.sync.dma_start(out=stats.ap(), in_=st_sb)

    _strip_self_waits(nc, mybir)
    _rebalance_waits(nc, mybir)
    nc.finalize()
    return nc


def _rebalance_waits(nc, mybir, max_waits=1, max_passes=256):
    """Push excess sync waits onto the preceding same-engine instruction.

    Engine queues are in-order, so hoisting a wait one slot earlier in
    the same engine's stream is strictly stronger and deadlock-free as
    long as the wait's producer doesn't depend on the hopped-over
    instruction (true for this kernel's slot-recycle waits, which
    reference work several tiles older). Same-semaphore waits merge by
    max value.
    """
    for func in nc.m.functions:
        for block in func.blocks:
            insts = [
                i
                for i in block.instructions
                if i.sync_info is not None or True
            ]
            streams = {}
            for i in insts:
                streams.setdefault(str(i.engine), []).append(i)
            for eng, stream in streams.items():
                for _ in range(max_passes):
                    moved = False
                    for idx in range(len(stream) - 1, 0, -1):
                        inst = stream[idx]
                        si = inst.sync_info
                        if si is None or len(si.on_wait) <= max_waits:
                            continue
                        waits = sorted(
                            si.on_wait, key=lambda w: w.wait_value
                        )
                        keep, excess = waits[max_waits:], waits[:max_waits]
                        # keep the newest on this inst, hoist the oldest
                        keep, excess = (
                            waits[len(waits) - max_waits :],
                            waits[: len(waits) - max_waits],
                        )
                        inst.sync_info = mybir.SyncInfo(
                            on_wait=keep, on_update=si.on_update
                        )
                        prev = stream[idx - 1]
                        psi = prev.sync_info or mybir.SyncInfo(
                            on_wait=[], on_update=[]
                        )
                        merged = {w.ant_name: w for w in psi.on_wait}
                        for w in excess:
                            cur = merged.get(w.ant_name)
                            if cur is None or w.wait_value > cur.wait_value:
                                merged[w.ant_name] = w
                        prev.sync_info = mybir.SyncInfo(
                            on_wait=list(merged.values()),
                            on_update=psi.on_update,
                        )
                        moved = True
                    if not moved:
                        break
            # Anything still over budget (e.g. the kernel-tail drain that
            # waits on every proc) gets a chain of single-wait drains
            # inserted just before it on the same engine.
            changed = False
            new_insts = []
            for inst in list(block.instructions):
                si = inst.sync_info
                if si is not None and len(si.on_wait) > max_waits:
                    waits = list(si.on_wait)
                    keep = waits[: max_waits]
                    for j, w in enumerate(waits[max_waits:]):
                        d = mybir.InstDrain(
                            name=f"{inst.name}-wsplit{j}",
                            ins=[],
                            outs=[],
                            bass_is_fusable=False,
                        )
                        d.engine = inst.engine
                        d.sync_info = mybir.SyncInfo(
                            on_wait=[w], on_update=[]
                        )
                        new_insts.append(d)
                        changed = True
                    inst.sync_info = mybir.SyncInfo(
                        on_wait=keep, on_update=si.on_update
                    )
                new_insts.append(inst)
            if changed:
                try:
                    block.instructions = new_insts
                except (AttributeError, TypeError):
                    block.instructions.clear()
                    block.instructions.extend(new_insts)


def _strip_self_waits(nc, mybir):
    """Drop same-engine semaphore waits (PE waiting on PE, etc).

    Engine queues execute in order, so a wait on the instruction's own
    engine semaphore is redundant at runtime; Tile emits them
    conservatively for slot-recycle WAW hazards, but this walrus build
    only allows one sync wait per instruction. DMA-queue semaphores are
    never touched.
    """
    compute = ("PE", "Activation", "DVE", "Pool", "SP")
    for inst in nc.inst_map.values():
        si = inst.sync_info
        if si is None or not si.on_wait:
            continue
        prefix = str(inst.engine).split(".")[-1] + "_"
        if not prefix.startswith(compute):
            continue
        kept = [w for w in si.on_wait if not w.ant_name.startswith(prefix)]
        if len(kept) != len(si.on_wait):
            inst.sync_info = mybir.SyncInfo(on_wait=kept, on_update=si.on_update)


def check_waits(nc, max_waits=1):
    """Count instructions exceeding the per-instruction sync-wait budget."""
    bad = []
    for name, inst in nc.inst_map.items():
        si = inst.sync_info
        if si is not None and len(si.on_wait) > max_waits:
            bad.append(
                (
                    name,
                    type(inst).__name__,
                    [(w.ant_name, w.wait_value) for w in si.on_wait],
                )
            )
    return bad


def kernel(x: np.ndarray, y: np.ndarray) -> np.ndarray:
    from concourse.bass_utils import run_bass_kernel_spmd

    x = np.asarray(x, dtype=np.float32)
    y = np.asarray(y, dtype=np.float32)

    # host-side prep: layout + tiny O(N*K) row stats
    x2 = np.einsum("ij,ij->i", x, x)                      # [N]
    y2 = np.einsum("ij,ij->i", y, y)                      # [M]
    ey_row = np.exp(-0.5 * y2).astype(ml_dtypes.bfloat16)  # [M]
    ey_rep = np.ascontiguousarray(np.broadcast_to(ey_row, (P, M)))
    yt = np.ascontiguousarray(y.T.astype(ml_dtypes.bfloat16))  # [K, M]
    xt_full = np.ascontiguousarray(x.T.astype(ml_dtypes.bfloat16))  # [K, N]

    in_maps = []
    for c in range(NCORES):
        sl = slice(c * MPC, (c + 1) * MPC)
        xb_c = np.ascontiguousarray(
            (-0.5 * x2[sl]).astype(np.float32).reshape(MB, P).T
        )
        in_maps.append(
            {
                "xt": np.ascontiguousarray(xt_full[:, sl]),
                "yt": yt,
                "xb": xb_c,
                "ey": ey_rep,
            }
        )

    if "nc" not in _cached:
        _cached["nc"] = _build()
    global _last_in_maps
    _last_in_maps = in_maps
    res = run_bass_kernel_spmd(_cached["nc"], in_maps, core_ids=list(range(NCORES)))

    total = 0.0
    for r in res.results:
        total += r["stats"].astype(np.float64).sum()
    return np.float32(total / (float(N) * float(M)))



# revision 5
# speedup vs baseline: 4.7483x; 4.7483x over previous
"""Gaussian RBF kernel-mean loss on 8 Trainium2 NeuronCores.

Computes mean(exp(-||x_i - y_j||^2 / 2)) over all (i, j) pairs for
x, y of shape [8192, 256] fp32.

Math used on device (per core, rows of x sharded 1024/core):
    exp(-d2/2) = exp(x.y - 0.5||x||^2 - 0.5||y||^2)
The column term -0.5||y_n||^2 is folded into the contraction as two
error-compensated bf16 rows (c1 = bf16(v), c2 = bf16(v - c1)) appended
to y^T, multiplied by constant-1 rows on the x side, so
    psum = x.y + c1 + c2          # PE, fp32 accumulate
    E    = exp(psum + bias_m)     # ACT, bias is per-partition -0.5||x_m||^2
    stats[:, t] = sum_n E         # ACT accum_out, fp32
The host adds the 8 * 128 * NTILES partials and divides by N*M.

Distribution: the wall-clock cost of a call is dominated by shipping
bytes over the tunneled PJRT link, so each core receives only its own
x shard [256, 1024] and y shard [258, 1024] (~1MB bf16 total per core);
the full y^T is assembled ON DEVICE with a DRAM AllGather across the 8
cores. Total host->device traffic ~8.3MB vs 54.6MB for the replicated
layout.

Execution: the first call goes through bass_utils.run_bass_kernel_spmd
(which compiles the NEFF). Subsequent calls reuse a process-cached
jax.jit(shard_map) wrapper built on the same bass2jax primitives, which
skips the per-call retrace + walrus recompile that run_bass_kernel_spmd
pays (it constructs a fresh jit closure per invocation).

Toolchain constraint: this walrus build accepts at most ONE sync wait
per compute/DMA instruction. The kernel keeps a strict PE -> ACT
pipeline; slot-recycle and DMA-arrival waits are absorbed by tiny
same-engine "observer" ops (LDWEIGHTS on PE, a scalar warmup on ACT)
and the _strip_self_waits/_rebalance_waits postpasses below.
"""

import numpy as np
import ml_dtypes

N = 8192          # rows of x
M = 8192          # rows of y
K = 256           # feature dim
NCORES = 8
MPC = N // NCORES        # 1024 rows of x (and y) per core
P = 128                  # partitions
KO = K // P              # 2 k-chunks
KA = K + 2               # y shard rows incl c1, c2 compensation rows
MB = MPC // P            # 8 m-blocks per core
NG_W = 2048              # columns per psum tile (4 banks)
NG = M // NG_W           # 4 n-groups
NS_W = 512               # matmul free width (1 psum bank)
NS = NG_W // NS_W        # 4
NTILES = MB * NG         # 32 output tiles per core

_cached = {}
_last_in_maps = None


def _build(fix_waits=True):
    import concourse.bass as bass
    import concourse.tile as tile
    import concourse.mybir as mybir
    from contextlib import ExitStack

    fp32 = mybir.dt.float32
    bf16 = mybir.dt.bfloat16

    nc = bass.Bass(trn_type="TRN2")
    xt = nc.dram_tensor("xt", [K, MPC], bf16, kind="ExternalInput")
    yts = nc.dram_tensor("yts", [KA, MPC], bf16, kind="ExternalInput")
    xb = nc.dram_tensor("xb", [P, MB], fp32, kind="ExternalInput")
    stats = nc.dram_tensor("stats", [P, NTILES], fp32, kind="ExternalOutput")

    xt_v = xt.ap().rearrange("(ko p) m -> p ko m", p=P)

    with ExitStack() as ctx:
        tc = ctx.enter_context(tile.TileContext(nc))
        singles = ctx.enter_context(tc.tile_pool(name="singles", bufs=1))
        dram = ctx.enter_context(tc.tile_pool(name="dram", bufs=1, space="DRAM"))
        psum_pool = ctx.enter_context(
            tc.tile_pool(name="psum", bufs=2, space="PSUM")
        )
        e_pool = ctx.enter_context(tc.tile_pool(name="e", bufs=4))

        in_b = dram.tile([KA, MPC], bf16)
        ytg = dram.tile([NCORES, KA, MPC], bf16)

        xt_sb = singles.tile([P, KO, MPC], bf16)
        ytg_sb = singles.tile([P, KO, M], bf16)
        yaug_sb = singles.tile([2, M], bf16)
        ones_sb = singles.tile([2, P], bf16)
        xb_sb = singles.tile([P, MB], fp32)
        st_sb = singles.tile([P, NTILES], fp32)
        warm = singles.tile([P, 1], fp32)

        # y shard -> DRAM bounce -> AllGather to full y^T (+aug rows)
        nc.gpsimd.dma_start(out=in_b, in_=yts.ap())
        nc.gpsimd.collective_compute(
            "AllGather",
            mybir.AluOpType.bypass,
            replica_groups=[list(range(NCORES))],
            ins=[in_b.opt()],
            outs=[ytg.opt()],
        )

        nc.sync.dma_start(out=xt_sb, in_=xt_v)
        nc.sync.dma_start(out=xb_sb, in_=xb.ap())
        nc.vector.memset(ones_sb, 1.0)
        # PE observer for the xt DMA queue (no PSUM write -> no bank WAW)
        nc.tensor.ldweights(weights=xt_sb[:, 0, 0:P])
        # ACT warmup: loads the exp table set AND observes the xb DMA queue,
        # so no later Exp carries the table-load's extra sync wait.
        nc.scalar.activation(
            out=warm, in_=xb_sb[:, 0:1], func=mybir.ActivationFunctionType.Exp
        )
        # gathered y columns: per source core j, feature chunks + aug rows
        for j in range(NCORES):
            cs = slice(j * MPC, (j + 1) * MPC)
            for ko in range(KO):
                nc.sync.dma_start(
                    out=ytg_sb[:, ko, cs],
                    in_=ytg[j, ko * P : (ko + 1) * P, :],
                )
            nc.sync.dma_start(out=yaug_sb[:, cs], in_=ytg[j, K:KA, :])

        e_list = []
        t = 0
        for mb in range(MB):
            ms = slice(mb * P, (mb + 1) * P)
            for ng in range(NG):
                if mb == 0:
                    # PE observer: absorb the ytg shard DMA-arrival waits
                    j0 = ng * (NG_W // MPC)
                    c0 = j0 * MPC
                    nc.tensor.ldweights(weights=ytg_sb[:, 0, c0 : c0 + P])
                if t >= 2:
                    # PE observer: absorb the psum-slot-recycle wait
                    # (ACT finished exp of tile t-2).
                    nc.tensor.ldweights(weights=e_list[t - 2][:, 0:P])
                psum = psum_pool.tile([P, NG_W], fp32)
                for ns in range(NS):
                    c0 = ng * NG_W + ns * NS_W
                    out_sl = psum[:, ns * NS_W : (ns + 1) * NS_W]
                    nc.tensor.matmul(
                        out_sl,
                        xt_sb[:, 0, ms],
                        ytg_sb[:, 0, c0 : c0 + NS_W],
                        start=True,
                        stop=False,
                    )
                    nc.tensor.matmul(
                        out_sl,
                        xt_sb[:, 1, ms],
                        ytg_sb[:, 1, c0 : c0 + NS_W],
                        start=False,
                        stop=False,
                    )
                    nc.tensor.matmul(
                        out_sl,
                        ones_sb,
                        yaug_sb[:, c0 : c0 + NS_W],
                        start=False,
                        stop=True,
                    )
                e_t = e_pool.tile([P, NG_W], bf16)
                nc.scalar.activation(
                    out=e_t,
                    in_=psum,
                    func=mybir.ActivationFunctionType.Exp,
                    bias=xb_sb[:, mb : mb + 1],
                    scale=1.0,
                    accum_out=st_sb[:, t : t + 1],
                )
                e_list.append(e_t)
                t += 1

        nc.sync.dma_start(out=stats.ap(), in_=st_sb)

    if fix_waits:
        _strip_self_waits(nc, mybir)
        _rebalance_waits(nc, mybir)
    nc.finalize()
    return nc


def _rebalance_waits(nc, mybir, max_waits=1):
    """Split over-budget sync waits into single-wait same-engine drains.

    Any instruction with more than `max_waits` waits gets a chain of
    no-op InstDrain instructions inserted just before it on the same
    engine, each carrying one of the excess waits. Engine streams are
    in-order, so the drains gate the instruction exactly as the
    original multi-wait would, with no reordering of dependencies
    (unlike hoisting waits onto earlier instructions, which can
    deadlock when the hoist target gates the wait's producer).
    """
    for func in nc.m.functions:
        for block in func.blocks:
            changed = False
            new_insts = []
            for inst in list(block.instructions):
                si = inst.sync_info
                if si is not None and len(si.on_wait) > max_waits:
                    waits = list(si.on_wait)
                    keep = waits[:max_waits]
                    for j, w in enumerate(waits[max_waits:]):
                        d = mybir.InstDrain(
                            name=f"{inst.name}-wsplit{j}",
                            ins=[],
                            outs=[],
                            bass_is_fusable=False,
                        )
                        d.engine = inst.engine
                        d.sync_info = mybir.SyncInfo(
                            on_wait=[w], on_update=[]
                        )
                        new_insts.append(d)
                        changed = True
                    inst.sync_info = mybir.SyncInfo(
                        on_wait=keep, on_update=si.on_update
                    )
                new_insts.append(inst)
            if changed:
                try:
                    block.instructions = new_insts
                except (AttributeError, TypeError):
                    block.instructions.clear()
                    block.instructions.extend(new_insts)


def _strip_self_waits(nc, mybir):
    """Drop same-engine semaphore waits (PE waiting on PE, etc).

    Engine queues execute in order, so a wait on the instruction's own
    engine semaphore is redundant at runtime; Tile emits them
    conservatively for slot-recycle WAW hazards, but this walrus build
    only allows one sync wait per instruction. DMA-queue semaphores are
    never touched.
    """
    compute = ("PE", "Activation", "DVE", "Pool", "SP")
    for inst in nc.inst_map.values():
        si = inst.sync_info
        if si is None or not si.on_wait:
            continue
        prefix = str(inst.engine).split(".")[-1] + "_"
        if not prefix.startswith(compute):
            continue
        kept = [w for w in si.on_wait if not w.ant_name.startswith(prefix)]
        if len(kept) != len(si.on_wait):
            inst.sync_info = mybir.SyncInfo(on_wait=kept, on_update=si.on_update)


def check_waits(nc, max_waits=1):
    """Count instructions exceeding the per-instruction sync-wait budget."""
    bad = []
    for name, inst in nc.inst_map.items():
        si = inst.sync_info
        if si is not None and len(si.on_wait) > max_waits:
            bad.append(
                (
                    name,
                    type(inst).__name__,
                    [(w.ant_name, w.wait_value) for w in si.on_wait],
                )
            )
    return bad


def _prep(x, y):
    """Host-side layout: transposed bf16 shards + tiny O(N*K) row stats."""
    bf16 = ml_dtypes.bfloat16
    x = np.asarray(x, dtype=np.float32)
    y = np.asarray(y, dtype=np.float32)

    x2 = np.einsum("ij,ij->i", x, x)                      # [N]
    y2 = np.einsum("ij,ij->i", y, y)                      # [M]

    xt_g = x.reshape(NCORES, MPC, K).transpose(0, 2, 1).astype(bf16)
    cv = (-0.5 * y2).astype(np.float32)
    c1 = cv.astype(bf16)
    c2 = (cv - c1.astype(np.float32)).astype(bf16)
    yts_g = np.empty((NCORES, KA, MPC), bf16)
    yts_g[:, :K] = y.reshape(NCORES, MPC, K).transpose(0, 2, 1)
    yts_g[:, K] = c1.reshape(NCORES, MPC)
    yts_g[:, K + 1] = c2.reshape(NCORES, MPC)
    xb_g = np.ascontiguousarray(
        (-0.5 * x2).astype(np.float32).reshape(NCORES, MB, P).transpose(0, 2, 1)
    )
    return xt_g, yts_g, xb_g


def _build_fast_runner(nc):
    """Process-cached jit(shard_map) over the same bass2jax primitives
    run_bass_kernel_spmd uses, so repeat calls skip retrace + recompile."""
    import jax
    import numpy as jnp_np  # noqa: F401
    from jax.sharding import Mesh, PartitionSpec
    from jax.experimental.shard_map import shard_map
    import concourse.mybir as mybir
    from concourse.bass2jax import (
        _bass_exec_p,
        partition_id_tensor,
        install_neuronx_cc_hook,
    )

    install_neuronx_cc_hook()

    in_names, out_names, out_avals = [], [], []
    partition_name = (
        nc.partition_id_tensor.name if nc.partition_id_tensor else None
    )
    for alloc in nc.m.functions[0].allocations:
        if not isinstance(alloc, mybir.MemoryLocationSet):
            continue
        name = alloc.memorylocations[0].name
        if alloc.kind == "ExternalInput":
            if name != partition_name:
                in_names.append(name)
        elif alloc.kind == "ExternalOutput":
            out_names.append(name)
            shape = tuple(alloc.tensor_shape)
            dtype = mybir.dt.np(alloc.dtype)
            out_avals.append(jax.core.ShapedArray(shape, dtype))
    n_params = len(in_names)
    n_outs = len(out_avals)
    all_in_names = in_names + out_names + (
        [partition_name] if partition_name else []
    )
    donate = tuple(range(n_params, n_params + n_outs))

    def _body(*args):
        operands = list(args)
        if partition_name is not None:
            operands.append(partition_id_tensor())
        return tuple(
            _bass_exec_p.bind(
                *operands,
                out_avals=tuple(out_avals),
                in_names=tuple(all_in_names),
                out_names=tuple(out_names),
                lowering_input_output_aliases=(),
                sim_require_finite=True,
                sim_require_nnan=True,
                nc=nc,
            )
        )

    devices = jax.devices()[:NCORES]
    mesh = Mesh(np.asarray(devices), ("core",))
    sharded = jax.jit(
        shard_map(
            _body,
            mesh=mesh,
            in_specs=(PartitionSpec("core"),) * (n_params + n_outs),
            out_specs=(PartitionSpec("core"),) * n_outs,
            check_rep=False,
        ),
        donate_argnums=donate,
        keep_unused=True,
    )
    return sharded, in_names, out_names, out_avals


def kernel(x: np.ndarray, y: np.ndarray) -> np.ndarray:
    from concourse.bass_utils import run_bass_kernel_spmd

    xt_g, yts_g, xb_g = _prep(x, y)

    if "nc" not in _cached:
        _cached["nc"] = _build()
    nc = _cached["nc"]

    in_by_name = {"xt": xt_g, "yts": yts_g, "xb": xb_g}

    if "fast" not in _cached:
        # First call: compile + run through bass_utils.run_bass_kernel_spmd.
        in_maps = [
            {k: v[c] for k, v in in_by_name.items()} for c in range(NCORES)
        ]
        global _last_in_maps
        _last_in_maps = in_maps
        res = run_bass_kernel_spmd(
            nc, in_maps, core_ids=list(range(NCORES))
        )
        stats = np.stack([r["stats"] for r in res.results])
        try:
            _cached["fast"] = _build_fast_runner(nc)
        except Exception:
            _cached["fast"] = None
    else:
        fast = _cached["fast"]
        if fast is None:
            in_maps = [
                {k: v[c] for k, v in in_by_name.items()}
                for c in range(NCORES)
            ]
            res = run_bass_kernel_spmd(
                nc, in_maps, core_ids=list(range(NCORES))
            )
            stats = np.stack([r["stats"] for r in res.results])
        else:
            sharded, in_names, out_names, out_avals = fast
            concat_in = [
                np.ascontiguousarray(in_by_name[n]).reshape(
                    NCORES * in_by_name[n].shape[1], *in_by_name[n].shape[2:]
                )
                for n in in_names
            ]
            concat_zeros = [
                np.zeros((NCORES * a.shape[0], *a.shape[1:]), a.dtype)
                for a in out_avals
            ]
            outs = sharded(*concat_in, *concat_zeros)
            stats = np.asarray(outs[out_names.index("stats")]).reshape(
                NCORES, P, NTILES
            )

    total = stats.astype(np.float64).sum()
    return np.float32(total / (float(N) * float(M)))


# revision 12
# speedup vs baseline: 5.6631x; 1.1927x over previous
"""Gaussian RBF kernel-mean loss on 8 Trainium2 NeuronCores.

Computes mean(exp(-||x_i - y_j||^2 / 2)) over all (i, j) pairs for
x, y of shape [8192, 256] fp32.

Math used on device (per core, rows of x sharded 1024/core):
    exp(-d2/2) = exp(x.y - 0.5||x||^2 - 0.5||y||^2)
The column term -0.5||y_n||^2 is folded into the contraction as two
error-compensated bf16 rows (c1 = bf16(v), c2 = bf16(v - c1)) appended
to y^T, multiplied by constant-1 rows on the x side, so
    psum = x.y + c1 + c2          # PE, fp32 accumulate
    E    = exp(psum + bias_m)     # ACT, bias is per-partition -0.5||x_m||^2
    stats[:, t] = sum_n E         # ACT accum_out, fp32
The host adds the 8 * 128 * NTILES partials and divides by N*M.

Distribution: the wall-clock cost of a call is dominated by shipping
bytes over the tunneled PJRT link, so each core receives only its own
x shard [256, 1024] and y shard [258, 1024] (~1MB bf16 total per core);
the full y^T is assembled ON DEVICE with a DRAM AllGather across the 8
cores. Total host->device traffic ~8.3MB vs 54.6MB for the replicated
layout.

Execution: the first call goes through bass_utils.run_bass_kernel_spmd
(which compiles the NEFF). Subsequent calls reuse a process-cached
jax.jit(shard_map) wrapper built on the same bass2jax primitives, which
skips the per-call retrace + walrus recompile that run_bass_kernel_spmd
pays (it constructs a fresh jit closure per invocation).

Toolchain constraint: this walrus build accepts at most ONE sync wait
per compute/DMA instruction. The kernel keeps a strict PE -> ACT
pipeline; slot-recycle and DMA-arrival waits are absorbed by tiny
same-engine "observer" ops (LDWEIGHTS on PE, a scalar warmup on ACT)
and the _strip_self_waits/_rebalance_waits postpasses below.
"""

import numpy as np
import ml_dtypes

N = 8192          # rows of x
M = 8192          # rows of y
K = 256           # feature dim
NCORES = 8
MPC = N // NCORES        # 1024 rows of x (and y) per core
P = 128                  # partitions
KO = K // P              # 2 k-chunks
KA = K + 2               # y shard rows incl c1, c2 compensation rows
MB = MPC // P            # 8 m-blocks per core
NG_W = 2048              # columns per psum tile (4 banks)
NG = M // NG_W           # 4 n-groups
NS_W = 512               # matmul free width (1 psum bank)
NS = NG_W // NS_W        # 4
NTILES = MB * NG         # 32 output tiles per core

_cached = {}
_last_in_maps = None


def _build(fix_waits=True):
    import concourse.bass as bass
    import concourse.tile as tile
    import concourse.mybir as mybir
    from contextlib import ExitStack

    fp32 = mybir.dt.float32
    bf16 = mybir.dt.bfloat16
    f8 = mybir.dt.float8e4

    nc = bass.Bass(trn_type="TRN2")
    xt = nc.dram_tensor("xt", [K, MPC], f8, kind="ExternalInput")
    yf = nc.dram_tensor("yf", [K, MPC], f8, kind="ExternalInput")
    ya = nc.dram_tensor("ya", [2, MPC], bf16, kind="ExternalInput")
    xb = nc.dram_tensor("xb", [P, MB + 1], fp32, kind="ExternalInput")
    stats = nc.dram_tensor("stats", [P, 1], fp32, kind="ExternalOutput")

    xt_v = xt.ap().rearrange("(ko p) m -> p ko m", p=P)

    with ExitStack() as ctx:
        tc = ctx.enter_context(tile.TileContext(nc))
        singles = ctx.enter_context(tc.tile_pool(name="singles", bufs=1))
        dram = ctx.enter_context(tc.tile_pool(name="dram", bufs=1, space="DRAM"))
        psum_pool = ctx.enter_context(
            tc.tile_pool(name="psum", bufs=2, space="PSUM")
        )
        e_pool = ctx.enter_context(tc.tile_pool(name="e", bufs=4))

        in_f = dram.tile([K, MPC], f8)
        in_a = dram.tile([2, MPC], bf16)
        ytg = dram.tile([NCORES, K, MPC], f8)
        ytga = dram.tile([NCORES, 2, MPC], bf16)

        xt_sb = singles.tile([P, KO, MPC], f8)
        ytg_sb = singles.tile([P, KO, M], f8)
        yaug_sb = singles.tile([2, M], bf16)
        ones_sb = singles.tile([2, P], bf16)
        xb_sb = singles.tile([P, MB + 1], fp32)
        st_sb = singles.tile([P, NTILES], fp32)
        red_sb = singles.tile([P, 1], fp32)
        warm = singles.tile([P, 1], fp32)

        # y shard -> DRAM bounce -> AllGather to full y^T (+aug rows)
        nc.gpsimd.dma_start(out=in_f, in_=yf.ap())
        nc.gpsimd.dma_start(out=in_a, in_=ya.ap())
        nc.gpsimd.collective_compute(
            "AllGather",
            mybir.AluOpType.bypass,
            replica_groups=[list(range(NCORES))],
            ins=[in_f.opt()],
            outs=[ytg.opt()],
        )
        nc.gpsimd.collective_compute(
            "AllGather",
            mybir.AluOpType.bypass,
            replica_groups=[list(range(NCORES))],
            ins=[in_a.opt()],
            outs=[ytga.opt()],
        )

        nc.sync.dma_start(out=xt_sb, in_=xt_v)
        nc.sync.dma_start(out=xb_sb, in_=xb.ap())
        nc.vector.memset(ones_sb, 1.0)
        # PE observer for the xt DMA queue (no PSUM write -> no bank WAW)
        nc.tensor.ldweights(weights=xt_sb[:, 0, 0:P])
        # ACT warmup: loads the exp table set AND observes the xb DMA queue,
        # so no later Exp carries the table-load's extra sync wait.
        nc.scalar.activation(
            out=warm, in_=xb_sb[:, 0:1], func=mybir.ActivationFunctionType.Exp
        )
        # gathered y columns: per source core j, feature chunks + aug rows
        for j in range(NCORES):
            cs = slice(j * MPC, (j + 1) * MPC)
            for ko in range(KO):
                nc.sync.dma_start(
                    out=ytg_sb[:, ko, cs],
                    in_=ytg[j, ko * P : (ko + 1) * P, :],
                )
            nc.sync.dma_start(out=yaug_sb[:, cs], in_=ytga[j])

        e_list = []
        t = 0
        for mb in range(MB):
            ms = slice(mb * P, (mb + 1) * P)
            for ng in range(NG):
                if mb == 0:
                    # PE observer: absorb the ytg shard DMA-arrival waits
                    j0 = ng * (NG_W // MPC)
                    c0 = j0 * MPC
                    nc.tensor.ldweights(weights=ytg_sb[:, 0, c0 : c0 + P])
                if t >= 2:
                    # PE observer: absorb the psum-slot-recycle wait
                    # (ACT finished exp of tile t-2).
                    nc.tensor.ldweights(weights=e_list[t - 2][:, 0:P])
                psum = psum_pool.tile([P, NG_W], fp32)
                for ns in range(NS):
                    c0 = ng * NG_W + ns * NS_W
                    out_sl = psum[:, ns * NS_W : (ns + 1) * NS_W]
                    nc.tensor.matmul(
                        out_sl,
                        xt_sb[:, 0, ms],
                        ytg_sb[:, 0, c0 : c0 + NS_W],
                        start=True,
                        stop=False,
                    )
                    nc.tensor.matmul(
                        out_sl,
                        xt_sb[:, 1, ms],
                        ytg_sb[:, 1, c0 : c0 + NS_W],
                        start=False,
                        stop=False,
                    )
                    nc.tensor.matmul(
                        out_sl,
                        ones_sb,
                        yaug_sb[:, c0 : c0 + NS_W],
                        start=False,
                        stop=True,
                    )
                e_t = e_pool.tile([P, NG_W], bf16)
                nc.scalar.activation(
                    out=e_t,
                    in_=psum,
                    func=mybir.ActivationFunctionType.Exp,
                    bias=xb_sb[:, mb : mb + 1],
                    scale=xb_sb[:, MB : MB + 1],
                    accum_out=st_sb[:, t : t + 1],
                )
                e_list.append(e_t)
                t += 1

        nc.vector.tensor_reduce(
            out=red_sb,
            in_=st_sb,
            axis=mybir.AxisListType.X,
            op=mybir.AluOpType.add,
        )
        nc.sync.dma_start(out=stats.ap(), in_=red_sb)

    if fix_waits:
        _strip_self_waits(nc, mybir)
        _rebalance_waits(nc, mybir)
    nc.finalize()
    return nc


def _rebalance_waits(nc, mybir, max_waits=1):
    """Split over-budget sync waits into single-wait same-engine drains.

    Any instruction with more than `max_waits` waits gets a chain of
    no-op InstDrain instructions inserted just before it on the same
    engine, each carrying one of the excess waits. Engine streams are
    in-order, so the drains gate the instruction exactly as the
    original multi-wait would, with no reordering of dependencies
    (unlike hoisting waits onto earlier instructions, which can
    deadlock when the hoist target gates the wait's producer).
    """
    for func in nc.m.functions:
        for block in func.blocks:
            changed = False
            new_insts = []
            for inst in list(block.instructions):
                si = inst.sync_info
                if si is not None and len(si.on_wait) > max_waits:
                    waits = list(si.on_wait)
                    keep = waits[:max_waits]
                    for j, w in enumerate(waits[max_waits:]):
                        d = mybir.InstDrain(
                            name=f"{inst.name}-wsplit{j}",
                            ins=[],
                            outs=[],
                            bass_is_fusable=False,
                        )
                        d.engine = inst.engine
                        d.sync_info = mybir.SyncInfo(
                            on_wait=[w], on_update=[]
                        )
                        new_insts.append(d)
                        changed = True
                    inst.sync_info = mybir.SyncInfo(
                        on_wait=keep, on_update=si.on_update
                    )
                new_insts.append(inst)
            if changed:
                try:
                    block.instructions = new_insts
                except (AttributeError, TypeError):
                    block.instructions.clear()
                    block.instructions.extend(new_insts)


def _strip_self_waits(nc, mybir):
    """Drop same-engine semaphore waits (PE waiting on PE, etc).

    Engine queues execute in order, so a wait on the instruction's own
    engine semaphore is redundant at runtime; Tile emits them
    conservatively for slot-recycle WAW hazards, but this walrus build
    only allows one sync wait per instruction. DMA-queue semaphores are
    never touched.
    """
    compute = ("PE", "Activation", "DVE", "Pool", "SP")
    for inst in nc.inst_map.values():
        si = inst.sync_info
        if si is None or not si.on_wait:
            continue
        prefix = str(inst.engine).split(".")[-1] + "_"
        if not prefix.startswith(compute):
            continue
        kept = [w for w in si.on_wait if not w.ant_name.startswith(prefix)]
        if len(kept) != len(si.on_wait):
            inst.sync_info = mybir.SyncInfo(on_wait=kept, on_update=si.on_update)


def check_waits(nc, max_waits=1):
    """Count instructions exceeding the per-instruction sync-wait budget."""
    bad = []
    for name, inst in nc.inst_map.items():
        si = inst.sync_info
        if si is not None and len(si.on_wait) > max_waits:
            bad.append(
                (
                    name,
                    type(inst).__name__,
                    [(w.ant_name, w.wait_value) for w in si.on_wait],
                )
            )
    return bad


def _prep(x, y):
    """Host-side layout: scaled fp8 feature shards + tiny O(N*K) row stats.

    Features ship as s*x, s*y in fp8 e4m3 (s sized so the rms lands at 16,
    well inside fp8's normal range); the fp32-accurate psum is rescaled on
    ACT via scale=1/s^2 shipped in xb's last column. The y-column term
    ships as two error-compensated bf16 rows computed from the SCALED y,
    so scale*(s^2 x.y + c1 + c2) = x.y - 0.5||y||^2 to ~fp32 accuracy.
    """
    bf16 = ml_dtypes.bfloat16
    f8 = ml_dtypes.float8_e4m3
    x = np.asarray(x, dtype=np.float32)
    y = np.asarray(y, dtype=np.float32)

    x2 = np.einsum("ij,ij->i", x, x)                      # [N]
    y2 = np.einsum("ij,ij->i", y, y)                      # [M]

    rms2 = (x2.mean() + y2.mean()) / (2.0 * K)
    amax = float(max(x.max(), -x.min(), y.max(), -y.min(), 1e-30))
    s = min(16.0 / np.sqrt(max(rms2, 1e-30)), 200.0 / amax)
    s = np.float32(s)
    inv_s2 = np.float32(1.0) / (s * s)

    xt_g = (x.reshape(NCORES, MPC, K).transpose(0, 2, 1) * s).astype(f8)
    yf_g = (y.reshape(NCORES, MPC, K).transpose(0, 2, 1) * s).astype(f8)
    cv = (-0.5 * (s * s) * y2).astype(np.float32)
    c1 = cv.astype(bf16)
    c2 = (cv - c1.astype(np.float32)).astype(bf16)
    ya_g = np.empty((NCORES, 2, MPC), bf16)
    ya_g[:, 0] = c1.reshape(NCORES, MPC)
    ya_g[:, 1] = c2.reshape(NCORES, MPC)
    xb_g = np.empty((NCORES, P, MB + 1), np.float32)
    xb_g[:, :, :MB] = (-0.5 * x2).reshape(NCORES, MB, P).transpose(0, 2, 1)
    xb_g[:, :, MB] = inv_s2
    return {"xt": xt_g, "yf": yf_g, "ya": ya_g, "xb": xb_g}


def _build_fast_runner(nc):
    """Process-cached jit(shard_map) over the same bass2jax primitives
    run_bass_kernel_spmd uses, so repeat calls skip retrace + recompile."""
    import jax
    import numpy as jnp_np  # noqa: F401
    from jax.sharding import Mesh, PartitionSpec
    from jax.experimental.shard_map import shard_map
    import concourse.mybir as mybir
    from concourse.bass2jax import (
        _bass_exec_p,
        partition_id_tensor,
        install_neuronx_cc_hook,
    )

    install_neuronx_cc_hook()

    in_names, out_names, out_avals = [], [], []
    partition_name = (
        nc.partition_id_tensor.name if nc.partition_id_tensor else None
    )
    for alloc in nc.m.functions[0].allocations:
        if not isinstance(alloc, mybir.MemoryLocationSet):
            continue
        name = alloc.memorylocations[0].name
        if alloc.kind == "ExternalInput":
            if name != partition_name:
                in_names.append(name)
        elif alloc.kind == "ExternalOutput":
            out_names.append(name)
            shape = tuple(alloc.tensor_shape)
            dtype = mybir.dt.np(alloc.dtype)
            out_avals.append(jax.core.ShapedArray(shape, dtype))
    n_params = len(in_names)
    n_outs = len(out_avals)
    all_in_names = in_names + out_names + (
        [partition_name] if partition_name else []
    )
    donate = tuple(range(n_params, n_params + n_outs))

    def _body(*args):
        operands = list(args)
        if partition_name is not None:
            operands.append(partition_id_tensor())
        return tuple(
            _bass_exec_p.bind(
                *operands,
                out_avals=tuple(out_avals),
                in_names=tuple(all_in_names),
                out_names=tuple(out_names),
                lowering_input_output_aliases=(),
                sim_require_finite=True,
                sim_require_nnan=True,
                nc=nc,
            )
        )

    devices = jax.devices()[:NCORES]
    mesh = Mesh(np.asarray(devices), ("core",))
    sharded = jax.jit(
        shard_map(
            _body,
            mesh=mesh,
            in_specs=(PartitionSpec("core"),) * (n_params + n_outs),
            out_specs=(PartitionSpec("core"),) * n_outs,
            check_rep=False,
        ),
        donate_argnums=donate,
        keep_unused=True,
    )
    return sharded, in_names, out_names, out_avals


def kernel(x: np.ndarray, y: np.ndarray) -> np.ndarray:
    from concourse.bass_utils import run_bass_kernel_spmd

    in_by_name = _prep(x, y)

    if "nc" not in _cached:
        _cached["nc"] = _build()
    nc = _cached["nc"]

    if "fast" not in _cached:
        # First call: compile + run through bass_utils.run_bass_kernel_spmd.
        in_maps = [
            {k: v[c] for k, v in in_by_name.items()} for c in range(NCORES)
        ]
        global _last_in_maps
        _last_in_maps = in_maps
        res = run_bass_kernel_spmd(
            nc, in_maps, core_ids=list(range(NCORES))
        )
        stats = np.stack([r["stats"] for r in res.results])
        try:
            _cached["fast"] = _build_fast_runner(nc)
        except Exception:
            _cached["fast"] = None
    else:
        fast = _cached["fast"]
        if fast is None:
            in_maps = [
                {k: v[c] for k, v in in_by_name.items()}
                for c in range(NCORES)
            ]
            res = run_bass_kernel_spmd(
                nc, in_maps, core_ids=list(range(NCORES))
            )
            stats = np.stack([r["stats"] for r in res.results])
        else:
            sharded, in_names, out_names, out_avals = fast
            concat_in = [
                np.ascontiguousarray(in_by_name[n]).reshape(
                    NCORES * in_by_name[n].shape[1], *in_by_name[n].shape[2:]
                )
                for n in in_names
            ]
            concat_zeros = [
                np.zeros((NCORES * a.shape[0], *a.shape[1:]), a.dtype)
                for a in out_avals
            ]
            outs = sharded(*concat_in, *concat_zeros)
            stats = np.asarray(outs[out_names.index("stats")])

    total = stats.astype(np.float64).sum()
    return np.float32(total / (float(N) * float(M)))
